# revision 52
# baseline (speedup 1.0000x reference)
"""Trainium2 Bass kernel for nn_CrossAttention (B=2, N=2048, M=256, C=1024, H=16).

Sharding: 8 cores = 2 batches x 4 head-groups (4 heads each).
v3: denominator matmuls eliminated:
 - vS extended to 65 cols/head (65th = ones) so each AV matmul emits the
   softmax denominator as psum row 64 for free
 - per-head AV accumulators [65, NT] (4 psum banks)
 - head pairs repacked to [128, NT] via identity matmuls (J), denominators
   broadcast via K=1 sel matmuls, normalize with one DVE mul per pair
 - out-proj unchanged (K=128 over head pairs)
Host sums the 4 partials per batch and adds proj_b.
"""

import sys

sys.path.insert(0, "/opt/trn_rl_repo")

import numpy as np  # noqa: E402

import concourse.bass as bass  # noqa: E402
import concourse.tile as tile  # noqa: E402
from concourse import bacc, mybir  # noqa: E402
from concourse.bass_utils import run_bass_kernel_spmd  # noqa: E402

F32 = mybir.dt.float32
R32 = mybir.dt.float32r
BF16 = mybir.dt.bfloat16
F16 = mybir.dt.float16
AF = mybir.ActivationFunctionType
MUL = mybir.AluOpType.mult

H = 16
B = 2
N = 2048          # image tokens
M = 256           # text tokens
C = 1024
HD = 64           # head dim
EPS = 1e-6
S = N + M         # 2304 kv length
HPC = 4           # heads per core
NT = 512          # query tile
SCALE = HD ** -0.5
NCH = 18          # S // 128 kv chunks
VW = HPC * (HD + 1)   # 260: v width incl per-head ones column

_TCNT = [0]


def T(pool, shape, tag, bufs=None, dt=F32):
    _TCNT[0] += 1
    kw = dict(tag=tag, name=f"{tag}_{_TCNT[0]}")
    if bufs is not None:
        kw["bufs"] = bufs
    return pool.tile(shape, dt, **kw)


def build_program(loop_iters=None):
    nc = bacc.Bacc("TRN2", target_bir_lowering=False, debug=False)

    xT = nc.dram_tensor("xT", [C, N], BF16, kind="ExternalInput").ap()
    yT = nc.dram_tensor("yT", [C, M], BF16, kind="ExternalInput").ap()
    wqkvT = nc.dram_tensor("wqkvT", [C, 2 * HPC * HD], BF16, kind="ExternalInput").ap()
    bqkvr = nc.dram_tensor("bqkvr", [1, 4 * 128], BF16, kind="ExternalInput").ap()
    wkvT = nc.dram_tensor("wkvT", [C, HPC * HD], BF16, kind="ExternalInput").ap()
    wvxT = nc.dram_tensor("wvxT", [C, VW], BF16, kind="ExternalInput").ap()
    wvyT = nc.dram_tensor("wvyT", [C, VW], BF16, kind="ExternalInput").ap()
    bvx = nc.dram_tensor("bvx", [1, VW], R32, kind="ExternalInput").ap()
    bvy = nc.dram_tensor("bvy", [1, VW], R32, kind="ExternalInput").ap()
    ones1r = nc.dram_tensor("ones1r", [1, 128], R32, kind="ExternalInput").ap()
    bkvr = nc.dram_tensor("bkvr", [1, 2 * 128], BF16, kind="ExternalInput").ap()
    wproj2 = nc.dram_tensor("wproj2", [128, 2 * C], BF16, kind="ExternalInput").ap()
    onesb = nc.dram_tensor("onesb", [128, 2], R32, kind="ExternalInput").ap()
    w2qk = nc.dram_tensor("w2qk", [2, 2 * 128], R32, kind="ExternalInput").ap()
    jmat = nc.dram_tensor("jmat", [HD + 1, 2 * 128], BF16, kind="ExternalInput").ap()
    selr = nc.dram_tensor("selr", [HD + 1, 2 * 128], R32, kind="ExternalInput").ap()
    outT = nc.dram_tensor("outT", [C, N], F16, kind="ExternalOutput").ap()

    with tile.TileContext(nc) as tc:
        with (
            tc.tile_pool(name="const", bufs=1) as const,
            tc.tile_pool(name="sing", bufs=1) as sing,
        ):
            # DMA order matters: phase-1 needs yT/wkv/wvy first, then phase-2's
            # wqkv before the bulk wvx/wproj loads
            yT_sb = T(const, [128, 8, M], "yT", dt=BF16)
            nc.sync.dma_start(yT_sb, yT.rearrange("(o p) f -> p o f", p=128))
            wkv_sb = T(const, [128, 8, HPC * HD], "wkv", dt=BF16)
            nc.sync.dma_start(wkv_sb, wkvT.rearrange("(o p) f -> p o f", p=128))
            bkv_sb = T(const, [1, 2, 128], "bkvr", dt=BF16)
            nc.sync.dma_start(bkv_sb, bkvr.rearrange("p (a o) -> p a o", a=2))
            onesb_sb = T(const, [128, 2], "onesb", dt=R32)
            nc.sync.dma_start(onesb_sb, onesb)
            w2qk_sb = T(const, [2, 2, 128], "w2qk", dt=R32)
            nc.sync.dma_start(w2qk_sb, w2qk.rearrange("p (a o) -> p a o", a=2))
            wvy_sb = T(const, [128, 8, VW], "wvy", dt=BF16)
            nc.sync.dma_start(wvy_sb, wvyT.rearrange("(o p) f -> p o f", p=128))
            bvy_sb = T(const, [1, VW], "bvy", dt=R32)
            nc.sync.dma_start(bvy_sb, bvy)
            ones1_sb = T(const, [1, 128], "ones1r", dt=R32)
            nc.sync.dma_start(ones1_sb, ones1r)
            wqkv_sb = T(const, [128, 8, 2 * HPC * HD], "wqkv", dt=BF16)
            wqkv_r = wqkvT.rearrange("(o p) f -> p o f", p=128)
            for cc in range(8):
                nc.sync.dma_start(wqkv_sb[:, cc], wqkv_r[:, cc])
            bqkv_sb = T(const, [1, 4, 128], "bqkvr", dt=BF16)
            nc.sync.dma_start(bqkv_sb, bqkvr.rearrange("p (a o) -> p a o", a=4))
            # x resident in SBUF (bf16), one tile per query tile so the
            # (tile-granular) dependency tracker lets phase 2 start after
            # the first chunk lands
            xT_r = xT.rearrange("(o p) f -> p o f", p=128)
            xT_tiles = []
            xc0 = T(const, [128, 8, NT], "xT0", dt=BF16)
            xT_tiles.append(xc0)
            nc.sync.dma_start(xc0, xT_r[:, :, 0:NT])
            wvx_sb = T(const, [128, 8, VW], "wvx", dt=BF16)
            nc.sync.dma_start(wvx_sb, wvxT.rearrange("(o p) f -> p o f", p=128))
            bvx_sb = T(const, [1, VW], "bvx", dt=R32)
            nc.sync.dma_start(bvx_sb, bvx)
            for nt in range(1, N // NT):
                xcn = T(const, [128, 8, NT], f"xT{nt}", dt=BF16)
                xT_tiles.append(xcn)
                nc.sync.dma_start(xcn, xT_r[:, :, nt * NT : (nt + 1) * NT])
            jmat_sb = T(const, [HD + 1, 2, 128], "jmat", dt=BF16)
            nc.sync.dma_start(jmat_sb, jmat.rearrange("p (a o) -> p a o", a=2))
            selr_sb = T(const, [HD + 1, 2, 128], "selr", dt=R32)
            nc.sync.dma_start(selr_sb, selr.rearrange("p (a o) -> p a o", a=2))
            wproj_sb = T(const, [128, 2, C], "wproj", dt=BF16)
            nc.sync.dma_start(wproj_sb, wproj2.rearrange("p (a o) -> p a o", a=2))
            eps_sb = T(const, [128, 1], "epsc")
            nc.vector.memset(eps_sb, float(EPS))
            zero_sb = T(const, [128, 1], "zeroc")
            nc.vector.memset(zero_sb, 0.0)
            onerow_sb = T(const, [1, NT], "onerow", dt=BF16)
            nc.vector.memset(onerow_sb, 1.0)

            # persistent activations: channel-on-partition layouts
            qT = T(sing, [128, 2, N], "qT", dt=R32)       # [2 heads x 64d, hp, n]
            kT = T(sing, [128, 2, S], "kT", dt=R32)
            vS = T(sing, [128, NCH, VW], "vS", dt=BF16)   # [s%128, s//128, h*65+d]

            def norm_a(pool_ps, pool_wk, psum):
                """Stage A: rms stats of psum (bias already in psum) -> rmsv."""
                nsz = psum.shape[-1]
                sq = T(pool_wk, [128, NT], "w", dt=R32)[:, :nsz]
                # Act engine: DVE may read only one PSUM operand per op
                nc.scalar.activation(sq, psum, AF.Square)
                ssp = T(pool_ps, [2, NT], "paux", bufs=3)[:, :nsz]
                nc.tensor.matmul(ssp, onesb_sb, sq, start=True, stop=True)
                lnv = T(pool_wk, [2, NT], "w2", bufs=8)[:, :nsz]
                nc.scalar.activation(
                    lnv, ssp, AF.Ln, bias=eps_sb[0:2], scale=1.0 / HD
                )
                rmsv = T(pool_wk, [2, NT], "w2", bufs=8, dt=R32)[:, :nsz]
                nc.scalar.activation(rmsv, lnv, AF.Exp, bias=zero_sb[0:2],
                                     scale=-0.5)
                return rmsv

            def norm_b(pool_ps, pool_wk, psum, rmsv, wsel, dest):
                """Stage B: dest = psum * bcast(rmsv * w)  (w folded in lhsT)."""
                nsz = psum.shape[-1]
                rbc = T(pool_ps, [128, NT], "paux", bufs=3)[:, :nsz]
                nc.tensor.matmul(rbc, w2qk_sb[:, wsel], rmsv,
                                 start=True, stop=True)
                rbs = T(pool_wk, [128, NT], "w", dt=R32)[:, :nsz]
                nc.vector.tensor_copy(out=rbs, in_=rbc)
                nc.vector.tensor_mul(dest, psum, rbs)

            def v_proj(pool_ps, src_sb, t, w_sb, b_sb, j):
                """vS[:, j] = (src.T @ wv_ext + bv_ext) in [s, h*65+d] layout."""
                pv = T(pool_ps, [128, NT], "pmain", bufs=4)[:, :VW]
                for cc in range(8):
                    nc.tensor.matmul(
                        pv,
                        src_sb[:, cc, t * 128 : (t + 1) * 128],
                        w_sb[:, cc, :],
                        start=(cc == 0),
                        stop=False,
                    )
                nc.tensor.matmul(pv, ones1_sb, b_sb, start=False, stop=True)
                nc.vector.tensor_copy(out=vS[:, j, :], in_=pv)

            # ---- phase 1: KV projection of y (text tokens -> kv rows 2048..2303)
            import contextlib
            with contextlib.ExitStack() as _les:
                if loop_iters is not None:
                    _les.enter_context(tc.For_i(0, loop_iters, 1))
                with (
                    tc.tile_pool(name="pp12", bufs=3, space="PSUM") as pp12,
                    tc.tile_pool(name="wk", bufs=12) as wk,
                ):
                    # norm stages are emitted 1-2 blocks late so the PE
                    # stream never blocks in-order on the DVE/Act chain
                    from collections import deque
                    npend = deque()

                    def pace12():
                        if npend:
                            npend.popleft()()

                    def queue_norm(ps, wsel, dest):
                        stv = {}

                        def a():
                            stv["rmsv"] = norm_a(pp12, wk, ps)

                        def b():
                            norm_b(pp12, wk, ps, stv["rmsv"], wsel, dest)
                        npend.append(a)
                        npend.append(b)

                    for mc in range(2):  # [k01, k23]
                        ps = T(pp12, [128, NT], "pmain", bufs=4)[:, :M]
                        for cc in range(8):
                            nc.tensor.matmul(
                                ps,
                                wkv_sb[:, cc, mc * 128 : (mc + 1) * 128],
                                yT_sb[:, cc, :],
                                start=(cc == 0),
                                stop=False,
                            )
                        nc.tensor.matmul(ps, bkv_sb[:, mc], onerow_sb[:, :M],
                                         start=False, stop=True)
                        pace12()
                        queue_norm(ps, 1, kT[:, mc, N : N + M])
                    v_proj(pp12, yT_sb, 0, wvy_sb, bvy_sb, 16)
                    pace12()
                    v_proj(pp12, yT_sb, 1, wvy_sb, bvy_sb, 17)
                    pace12()

                    # ---- phase 2: QKV projection of x
                    for nt in range(N // NT):
                        nsl = slice(nt * NT, (nt + 1) * NT)
                        xc = xT_tiles[nt]
                        for mc in range(4):  # [q01,q23,k01,k23]
                            ps = T(pp12, [128, NT], "pmain", bufs=4)
                            for cc in range(8):
                                nc.tensor.matmul(
                                    ps,
                                    wqkv_sb[:, cc, mc * 128 : (mc + 1) * 128],
                                    xc[:, cc, :],
                                    start=(cc == 0),
                                    stop=False,
                                )
                            nc.tensor.matmul(ps, bqkv_sb[:, mc], onerow_sb,
                                             start=False, stop=True)
                            pace12()
                            if mc < 2:
                                queue_norm(ps, 0, qT[:, mc, nsl])
                            else:
                                queue_norm(ps, 1, kT[:, mc - 2, nsl])
                        for t in range(4):
                            v_proj(pp12, xc, t, wvx_sb, bvx_sb, nt * 4 + t)
                            pace12()
                    while npend:
                        npend.popleft()()

                # ---- phase 3+4: attention + output projection, per query tile
                with (
                    tc.tile_pool(name="pbig", bufs=1, space="PSUM") as pbig,
                    tc.tile_pool(name="pav", bufs=1, space="PSUM") as pav,
                    tc.tile_pool(name="paux", bufs=1, space="PSUM") as paux,
                    tc.tile_pool(name="atp", bufs=1) as atp,
                    tc.tile_pool(name="asp", bufs=1) as asp,
                    tc.tile_pool(name="osp", bufs=2) as osp,
                ):
                    def do_av(av_pair, hp, jg, ats):
                        for u in range(2):
                            j = 2 * jg + u
                            usl = slice(u * NT, (u + 1) * NT)
                            for idx in range(2):
                                h = 2 * hp + idx
                                nc.tensor.matmul(
                                    av_pair[idx][: HD + 1, :],
                                    vS[:, j, h * 65 : h * 65 + 65],
                                    ats[idx][:, usl],
                                    start=(j == 0), stop=(j == NCH - 1),
                                    skip_group_check=True,
                                )

                    from collections import deque
                    pending = deque()  # PE work paced into the next sweeps

                    def pace(k):
                        for _ in range(k):
                            if pending:
                                pending.popleft()()

                    for nt in range(N // NT):
                        nsl = slice(nt * NT, (nt + 1) * NT)
                        avc_l = []  # bf16 sbuf copies
                        rds_l = []  # f32 reciprocal rows
                        ot_l = [None, None]
                        st = {}

                        def mk_avp(hp, avc_l=avc_l, st=st):
                            def run():
                                avp = T(paux, [128, NT], "aux", bufs=2)
                                for q in range(2):
                                    nc.tensor.matmul(
                                        avp, jmat_sb[:, q], avc_l[2 * hp + q],
                                        start=(q == 0), stop=(q == 1),
                                    )
                                st[("avp", hp)] = avp
                            return run

                        def mk_bc(hp, rds_l=rds_l, ot_l=ot_l, st=st):
                            def run():
                                bcp = T(paux, [128, NT], "aux", bufs=2)
                                for q in range(2):
                                    nc.tensor.matmul(
                                        bcp,
                                        selr_sb[HD : HD + 1, q],
                                        rds_l[2 * hp + q][HD : HD + 1],
                                        start=(q == 0), stop=(q == 1),
                                        tile_position=(HD, 0),
                                    )
                                bcs = T(asp, [128, NT], "bcs", bufs=2)
                                nc.vector.tensor_copy(out=bcs, in_=bcp)
                                ot = T(asp, [128, NT], "ot", bufs=3, dt=BF16)
                                nc.vector.tensor_mul(ot, st[("avp", hp)], bcs)
                                ot_l[hp] = ot
                            return run

                        def mk_po(oc, ot_l=ot_l, st=st):
                            def run():
                                po = T(paux, [128, NT], "aux", bufs=2)
                                for p in range(2):
                                    nc.tensor.matmul(
                                        po,
                                        wproj_sb[:, p, oc * 128 : (oc + 1) * 128],
                                        ot_l[p],
                                        start=(p == 0), stop=(p == 1),
                                    )
                                st[("po", oc)] = po
                            return run

                        last_nt = nt == N // NT - 1

                        def mk_ob(ocp, nsl=nsl, st=st, use_act=last_nt):
                            def run():
                                ob = T(osp, [128, 2, NT], "ob", dt=F16)
                                if use_act:
                                    # tail flush: act engine is idle there
                                    nc.scalar.activation(
                                        ob[:, 0], st[("po", 2 * ocp)], AF.Copy
                                    )
                                    nc.scalar.activation(
                                        ob[:, 1], st[("po", 2 * ocp + 1)], AF.Copy
                                    )
                                else:
                                    nc.vector.tensor_copy(
                                        ob[:, 0], st[("po", 2 * ocp)]
                                    )
                                    nc.vector.tensor_copy(
                                        ob[:, 1], st[("po", 2 * ocp + 1)]
                                    )
                                nc.sync.dma_start(
                                    outT.rearrange("(o p) f -> p o f", p=128)[
                                        :, 2 * ocp : 2 * ocp + 2, nsl
                                    ],
                                    ob,
                                )
                            return run

                        for hp in range(2):
                            av_pair = [T(pav, [128, NT], "av", bufs=2)
                                       for _ in range(2)]
                            prev = None
                            for jg in range(9):  # 2-chunk batches, AV lags 1
                                pl = [T(pbig, [128, 2 * NT], "big", bufs=2)
                                      for _ in range(2)]
                                for u in range(2):
                                    j = 2 * jg + u
                                    for idx in range(2):
                                        prt = slice(64 * idx, 64 * idx + 64)
                                        nc.tensor.matmul(
                                            pl[idx][:, u * NT : (u + 1) * NT],
                                            kT[prt, hp, j * 128 : (j + 1) * 128],
                                            qT[prt, hp, nsl],
                                            start=True, stop=True,
                                            tile_position=(64 * idx, 0),
                                        )
                                ats = []
                                for idx in range(2):
                                    at = T(atp, [128, 2 * NT], "at", bufs=6,
                                           dt=BF16)
                                    nc.scalar.activation(
                                        at, pl[idx], AF.Exp,
                                        bias=zero_sb[:], scale=SCALE,
                                    )
                                    ats.append(at)
                                if prev is not None:
                                    do_av(av_pair, hp, prev[0], prev[1])
                                pace(1)
                                prev = (jg, ats)
                            do_av(av_pair, hp, prev[0], prev[1])
                            # psum -> sbuf + denominator reciprocals (overlap
                            # next pair's sweep)
                            for idx in range(2):
                                ac = T(asp, [HD + 1, NT], "avc", bufs=4, dt=BF16)
                                nc.vector.tensor_copy(
                                    out=ac, in_=av_pair[idx][: HD + 1, :]
                                )
                                avc_l.append(ac)
                                rd = T(asp, [HD + 1, NT], "rds", bufs=4, dt=R32)
                                with nc.allow_low_precision(
                                    reason="float32r output is fp32 storage"
                                ):
                                    nc.vector.reciprocal(
                                        rd[HD : HD + 1],
                                        av_pair[idx][HD : HD + 1, :],
                                    )
                                rds_l.append(rd)
                            if hp == 0:
                                # pair-0 normalize can run during pair-1 sweep
                                pending.append(mk_avp(0))
                                pending.append(mk_bc(0))

                        # queue pair-1 normalize + out-projection; they run
                        # interleaved with the next tile's QK sweeps
                        pending.append(mk_avp(1))
                        pending.append(mk_bc(1))
                        for oc in range(8):
                            pending.append(mk_po(oc))
                            if oc % 2 == 1:
                                pending.append(mk_ob(oc // 2))
                    while pending:
                        pending.popleft()()
    _orig = bacc.get_activation_tables

    def _tables(arch):
        t = _orig(arch)
        return {
            name: (set() if name in ("exp_and_others", "natural_log",
                                     "exp_and_friends") else fns)
            for name, fns in t.items()
        }

    bacc.get_activation_tables = _tables
    try:
        nc.compile()
    finally:
        bacc.get_activation_tables = _orig
    return nc


_PROGRAM = None


def _get_program():
    global _PROGRAM
    if _PROGRAM is None:
        _PROGRAM = build_program()
    return _PROGRAM


def _make_in_maps(x, y, qkv_w, qkv_b, kv_w, kv_b, qn_w, kn_w, proj_w, proj_b):
    import ml_dtypes

    f = np.float32
    bf = ml_dtypes.bfloat16
    onesb = np.zeros((128, 2), f)
    onesb[0:64, 0] = 1.0
    onesb[64:128, 1] = 1.0
    w2qk = np.zeros((2, 2, 128), f)
    for wsel, w in ((0, qn_w), (1, kn_w)):
        w2qk[0, wsel, 0:64] = w
        w2qk[1, wsel, 64:128] = w
    w2qk = w2qk.reshape(2, 256)
    jmat = np.zeros((HD + 1, 2, 128), f)
    for d in range(HD):
        jmat[d, 0, d] = 1.0
        jmat[d, 1, 64 + d] = 1.0
    jmat = jmat.reshape(HD + 1, 256).astype(bf)
    selr = np.zeros((HD + 1, 2, 128), f)
    selr[HD, 0, 0:64] = 1.0
    selr[HD, 1, 64:128] = 1.0
    selr = selr.reshape(HD + 1, 256)

    def _vext(w, b):
        """[256, C] weights + [256] bias -> [260, C] / [260] with per-head
        ones column (zero w row, bias 1.0)."""
        we = np.zeros((VW, C), f)
        be = np.zeros((VW,), f)
        for h in range(HPC):
            we[65 * h : 65 * h + 64] = w[64 * h : 64 * h + 64]
            be[65 * h : 65 * h + 64] = b[64 * h : 64 * h + 64]
            be[65 * h + 64] = 1.0
        return we, be

    in_maps = []
    for core in range(8):
        b, g = divmod(core, 4)
        qs = slice(g * 256, (g + 1) * 256)
        wqkv = np.concatenate([qkv_w[qs], qkv_w[1024:2048][qs]], axis=0)
        bq = np.concatenate([qkv_b[qs], qkv_b[1024:2048][qs]])
        wkv = kv_w[qs]
        bk = kv_b[qs]
        wvx, bvxv = _vext(qkv_w[2048:3072][qs], qkv_b[2048:3072][qs])
        wvy, bvyv = _vext(kv_w[1024:2048][qs], kv_b[1024:2048][qs])
        wp = np.ascontiguousarray(proj_w[:, qs].T, f)  # [256, 1024]
        wproj2 = np.ascontiguousarray(
            wp.reshape(2, 128, C).transpose(1, 0, 2).reshape(128, 2 * C)
        ).astype(bf)
        in_maps.append(
            {
                "xT": np.ascontiguousarray(x[b].T).astype(bf),
                "yT": np.ascontiguousarray(y[b].T).astype(bf),
                "wqkvT": np.ascontiguousarray(wqkv.T).astype(bf),
                "bqkvr": bq.reshape(1, 512).astype(bf),
                "wkvT": np.ascontiguousarray(wkv.T).astype(bf),
                "bkvr": bk.reshape(1, 256).astype(bf),
                "wvxT": np.ascontiguousarray(wvx.T).astype(bf),
                "bvx": np.ascontiguousarray(bvxv.reshape(1, VW), f),
                "wvyT": np.ascontiguousarray(wvy.T).astype(bf),
                "bvy": np.ascontiguousarray(bvyv.reshape(1, VW), f),
                "ones1r": np.ones((1, 128), f),
                "wproj2": wproj2,
                "onesb": onesb,
                "w2qk": w2qk,
                "jmat": jmat,
                "selr": selr,
            }
        )
    return in_maps


def run_cores(inputs, trace=False, **kwargs):
    nc = _get_program()
    in_maps = _make_in_maps(**{k: np.asarray(v, np.float32) for k, v in inputs.items()})
    return run_bass_kernel_spmd(
        nc, in_maps, core_ids=list(range(8)), trace=trace, **kwargs
    )


def kernel(**inputs):
    proj_b = np.asarray(inputs["proj_b"], np.float32)
    res = run_cores(inputs).results
    out = np.zeros((B, N, C), np.float32)
    for core in range(8):
        b = core // 4
        out[b] += res[core]["outT"].T.astype(np.float32)
    out += proj_b[None, None, :]
    return out


# revision 70
# speedup vs baseline: 6.8513x; 6.8513x over previous
"""Trainium2 Bass kernel for nn_CrossAttention (B=2, N=2048, M=256, C=1024, H=16).

Sharding: 8 cores = 2 batches x 4 head-groups (4 heads each).
v3: denominator matmuls eliminated:
 - vS extended to 65 cols/head (65th = ones) so each AV matmul emits the
   softmax denominator as psum row 64 for free
 - per-head AV accumulators [65, NT] (4 psum banks)
 - head pairs repacked to [128, NT] via identity matmuls (J), denominators
   broadcast via K=1 sel matmuls, normalize with one DVE mul per pair
 - out-proj unchanged (K=128 over head pairs)
Host sums the 4 partials per batch and adds proj_b.
"""

import sys

sys.path.insert(0, "/opt/trn_rl_repo")

import numpy as np  # noqa: E402

import concourse.bass as bass  # noqa: E402
import concourse.tile as tile  # noqa: E402
from concourse import bacc, mybir  # noqa: E402
from concourse.bass_utils import run_bass_kernel_spmd  # noqa: E402

F32 = mybir.dt.float32
R32 = mybir.dt.float32r
BF16 = mybir.dt.bfloat16
F16 = mybir.dt.float16
AF = mybir.ActivationFunctionType
MUL = mybir.AluOpType.mult

H = 16
B = 2
N = 2048          # image tokens
M = 256           # text tokens
C = 1024
HD = 64           # head dim
EPS = 1e-6
S = N + M         # 2304 kv length
HPC = 4           # heads per core
NT = 512          # query tile
SCALE = HD ** -0.5
NCH = 18          # S // 128 kv chunks
VW = HPC * (HD + 1)   # 260: v width incl per-head ones column

_TCNT = [0]


def T(pool, shape, tag, bufs=None, dt=F32):
    _TCNT[0] += 1
    kw = dict(tag=tag, name=f"{tag}_{_TCNT[0]}")
    if bufs is not None:
        kw["bufs"] = bufs
    return pool.tile(shape, dt, **kw)


def build_program(loop_iters=None):
    nc = bacc.Bacc("TRN2", target_bir_lowering=False, debug=False)

    xT = nc.dram_tensor("xT", [C, N], BF16, kind="ExternalInput").ap()
    yT = nc.dram_tensor("yT", [C, M], BF16, kind="ExternalInput").ap()
    wqkvT = nc.dram_tensor("wqkvT", [C, 2 * HPC * HD], BF16, kind="ExternalInput").ap()
    bqkvr = nc.dram_tensor("bqkvr", [1, 4 * 128], BF16, kind="ExternalInput").ap()
    wkvT = nc.dram_tensor("wkvT", [C, HPC * HD], BF16, kind="ExternalInput").ap()
    wvxT = nc.dram_tensor("wvxT", [C, VW], BF16, kind="ExternalInput").ap()
    wvyT = nc.dram_tensor("wvyT", [C, VW], BF16, kind="ExternalInput").ap()
    bvx = nc.dram_tensor("bvx", [1, VW], R32, kind="ExternalInput").ap()
    bvy = nc.dram_tensor("bvy", [1, VW], R32, kind="ExternalInput").ap()
    ones1r = nc.dram_tensor("ones1r", [1, 128], R32, kind="ExternalInput").ap()
    bkvr = nc.dram_tensor("bkvr", [1, 2 * 128], BF16, kind="ExternalInput").ap()
    wproj2 = nc.dram_tensor("wproj2", [128, 2 * C], BF16, kind="ExternalInput").ap()
    onesb = nc.dram_tensor("onesb", [128, 2], R32, kind="ExternalInput").ap()
    w2qk = nc.dram_tensor("w2qk", [2, 2 * 128], R32, kind="ExternalInput").ap()
    jmat = nc.dram_tensor("jmat", [HD + 1, 2 * 128], BF16, kind="ExternalInput").ap()
    selr = nc.dram_tensor("selr", [HD + 1, 2 * 128], R32, kind="ExternalInput").ap()
    outT = nc.dram_tensor("outT", [C, N], F16, kind="ExternalOutput").ap()
    # pair-0 half of the final query tile's out-proj (host adds it back)
    outT2 = nc.dram_tensor("outT2", [C, NT], F16, kind="ExternalOutput").ap()

    with tile.TileContext(nc) as tc:
        with (
            tc.tile_pool(name="const", bufs=1) as const,
            tc.tile_pool(name="sing", bufs=1) as sing,
        ):
            # DMA order matters: phase-1 needs yT/wkv/wvy first, then phase-2's
            # wqkv before the bulk wvx/wproj loads
            yT_sb = T(const, [128, 8, M], "yT", dt=BF16)
            nc.sync.dma_start(yT_sb, yT.rearrange("(o p) f -> p o f", p=128))
            wkv_sb = T(const, [128, 8, HPC * HD], "wkv", dt=BF16)
            nc.sync.dma_start(wkv_sb, wkvT.rearrange("(o p) f -> p o f", p=128))
            bkv_sb = T(const, [1, 2, 128], "bkvr", dt=BF16)
            nc.sync.dma_start(bkv_sb, bkvr.rearrange("p (a o) -> p a o", a=2))
            onesb_sb = T(const, [128, 2], "onesb", dt=R32)
            nc.sync.dma_start(onesb_sb, onesb)
            w2qk_sb = T(const, [2, 2, 128], "w2qk", dt=R32)
            nc.sync.dma_start(w2qk_sb, w2qk.rearrange("p (a o) -> p a o", a=2))
            wvy_sb = T(const, [128, 8, VW], "wvy", dt=BF16)
            nc.sync.dma_start(wvy_sb, wvyT.rearrange("(o p) f -> p o f", p=128))
            bvy_sb = T(const, [1, VW], "bvy", dt=R32)
            nc.sync.dma_start(bvy_sb, bvy)
            ones1_sb = T(const, [1, 128], "ones1r", dt=R32)
            nc.sync.dma_start(ones1_sb, ones1r)
            wqkv_sb = T(const, [128, 8, 2 * HPC * HD], "wqkv", dt=BF16)
            wqkv_r = wqkvT.rearrange("(o p) f -> p o f", p=128)
            for cc in range(8):
                nc.sync.dma_start(wqkv_sb[:, cc], wqkv_r[:, cc])
            bqkv_sb = T(const, [1, 4, 128], "bqkvr", dt=BF16)
            nc.sync.dma_start(bqkv_sb, bqkvr.rearrange("p (a o) -> p a o", a=4))
            # x resident in SBUF (bf16), one tile per query tile so the
            # (tile-granular) dependency tracker lets phase 2 start after
            # the first chunk lands
            xT_r = xT.rearrange("(o p) f -> p o f", p=128)
            xT_tiles = []
            xc0 = T(const, [128, 8, NT], "xT0", dt=BF16)
            xT_tiles.append(xc0)
            nc.sync.dma_start(xc0, xT_r[:, :, 0:NT])
            wvx_sb = T(const, [128, 8, VW], "wvx", dt=BF16)
            nc.sync.dma_start(wvx_sb, wvxT.rearrange("(o p) f -> p o f", p=128))
            bvx_sb = T(const, [1, VW], "bvx", dt=R32)
            nc.sync.dma_start(bvx_sb, bvx)
            for nt in range(1, N // NT):
                xcn = T(const, [128, 8, NT], f"xT{nt}", dt=BF16)
                xT_tiles.append(xcn)
                nc.sync.dma_start(xcn, xT_r[:, :, nt * NT : (nt + 1) * NT])
            jmat_sb = T(const, [HD + 1, 2, 128], "jmat", dt=BF16)
            nc.sync.dma_start(jmat_sb, jmat.rearrange("p (a o) -> p a o", a=2))
            selr_sb = T(const, [HD + 1, 2, 128], "selr", dt=R32)
            nc.sync.dma_start(selr_sb, selr.rearrange("p (a o) -> p a o", a=2))
            wproj_sb = T(const, [128, 2, C], "wproj", dt=BF16)
            nc.sync.dma_start(wproj_sb, wproj2.rearrange("p (a o) -> p a o", a=2))
            eps_sb = T(const, [128, 1], "epsc")
            nc.vector.memset(eps_sb, float(EPS))
            zero_sb = T(const, [128, 1], "zeroc")
            nc.vector.memset(zero_sb, 0.0)
            onerow_sb = T(const, [1, NT], "onerow", dt=BF16)
            nc.vector.memset(onerow_sb, 1.0)

            # persistent activations: channel-on-partition layouts
            qT = T(sing, [128, 2, N], "qT", dt=R32)       # [2 heads x 64d, hp, n]
            kT = T(sing, [128, 2, S], "kT", dt=R32)
            vS = T(sing, [128, NCH, VW], "vS", dt=BF16)   # [s%128, s//128, h*65+d]

            def norm_a(pool_ps, pool_wk, psum):
                """Stage A: rms stats of psum (bias already in psum) -> rmsv."""
                nsz = psum.shape[-1]
                sq = T(pool_wk, [128, NT], "w", dt=R32)[:, :nsz]
                # Act engine: DVE may read only one PSUM operand per op
                nc.scalar.activation(sq, psum, AF.Square)
                ssp = T(pool_ps, [2, NT], "paux", bufs=3)[:, :nsz]
                nc.tensor.matmul(ssp, onesb_sb, sq, start=True, stop=True)
                lnv = T(pool_wk, [2, NT], "w2", bufs=8)[:, :nsz]
                nc.scalar.activation(
                    lnv, ssp, AF.Ln, bias=eps_sb[0:2], scale=1.0 / HD
                )
                rmsv = T(pool_wk, [2, NT], "w2", bufs=8, dt=R32)[:, :nsz]
                nc.scalar.activation(rmsv, lnv, AF.Exp, bias=zero_sb[0:2],
                                     scale=-0.5)
                return rmsv

            def norm_b(pool_ps, pool_wk, psum, rmsv, wsel, dest):
                """Stage B: dest = psum * bcast(rmsv * w)  (w folded in lhsT)."""
                nsz = psum.shape[-1]
                rbc = T(pool_ps, [128, NT], "paux", bufs=3)[:, :nsz]
                nc.tensor.matmul(rbc, w2qk_sb[:, wsel], rmsv,
                                 start=True, stop=True)
                rbs = T(pool_wk, [128, NT], "w", dt=R32)[:, :nsz]
                nc.vector.tensor_copy(out=rbs, in_=rbc)
                nc.vector.tensor_mul(dest, psum, rbs)

            def v_proj(pool_ps, src_sb, t, w_sb, b_sb, j):
                """vS[:, j] = (src.T @ wv_ext + bv_ext) in [s, h*65+d] layout."""
                pv = T(pool_ps, [128, NT], "pmain", bufs=5)[:, :VW]
                for cc in range(8):
                    nc.tensor.matmul(
                        pv,
                        src_sb[:, cc, t * 128 : (t + 1) * 128],
                        w_sb[:, cc, :],
                        start=(cc == 0),
                        stop=False,
                    )
                nc.tensor.matmul(pv, ones1_sb, b_sb, start=False, stop=True)
                nc.vector.tensor_copy(out=vS[:, j, :], in_=pv)

            # ---- phase 1: KV projection of y (text tokens -> kv rows 2048..2303)
            import contextlib
            with contextlib.ExitStack() as _les:
                if loop_iters is not None:
                    _les.enter_context(tc.For_i(0, loop_iters, 1))
                with (
                    tc.tile_pool(name="pp12", bufs=3, space="PSUM") as pp12,
                    tc.tile_pool(name="wk", bufs=12) as wk,
                ):
                    # norm stages are emitted 1-2 blocks late so the PE
                    # stream never blocks in-order on the DVE/Act chain
                    from collections import deque
                    npend = deque()

                    def pace12():
                        if npend:
                            npend.popleft()()

                    def queue_norm(ps, wsel, dest):
                        stv = {}

                        def a():
                            stv["rmsv"] = norm_a(pp12, wk, ps)

                        def b():
                            norm_b(pp12, wk, ps, stv["rmsv"], wsel, dest)
                        npend.append(a)
                        npend.append(b)

                    for mc in range(2):  # [k01, k23]
                        ps = T(pp12, [128, NT], "pmain", bufs=5)[:, :M]
                        for cc in range(8):
                            nc.tensor.matmul(
                                ps,
                                wkv_sb[:, cc, mc * 128 : (mc + 1) * 128],
                                yT_sb[:, cc, :],
                                start=(cc == 0),
                                stop=False,
                            )
                        nc.tensor.matmul(ps, bkv_sb[:, mc], onerow_sb[:, :M],
                                         start=False, stop=True)
                        pace12()
                        queue_norm(ps, 1, kT[:, mc, N : N + M])
                    v_proj(pp12, yT_sb, 0, wvy_sb, bvy_sb, 16)
                    pace12()
                    v_proj(pp12, yT_sb, 1, wvy_sb, bvy_sb, 17)
                    pace12()

                    # ---- phase 2: QKV projection of x
                    for nt in range(N // NT):
                        nsl = slice(nt * NT, (nt + 1) * NT)
                        xc = xT_tiles[nt]
                        for mc in range(4):  # [q01,q23,k01,k23]
                            ps = T(pp12, [128, NT], "pmain", bufs=5)
                            for cc in range(8):
                                nc.tensor.matmul(
                                    ps,
                                    wqkv_sb[:, cc, mc * 128 : (mc + 1) * 128],
                                    xc[:, cc, :],
                                    start=(cc == 0),
                                    stop=False,
                                )
                            nc.tensor.matmul(ps, bqkv_sb[:, mc], onerow_sb,
                                             start=False, stop=True)
                            pace12()
                            if mc < 2:
                                queue_norm(ps, 0, qT[:, mc, nsl])
                            else:
                                queue_norm(ps, 1, kT[:, mc - 2, nsl])
                        for t in range(4):
                            v_proj(pp12, xc, t, wvx_sb, bvx_sb, nt * 4 + t)
                            pace12()
                            if nt == N // NT - 1:
                                pace12()
                    while npend:
                        npend.popleft()()

                # ---- phase 3+4: attention + output projection, per query tile
                with (
                    tc.tile_pool(name="pbig", bufs=1, space="PSUM") as pbig,
                    tc.tile_pool(name="pav", bufs=1, space="PSUM") as pav,
                    tc.tile_pool(name="paux", bufs=1, space="PSUM") as paux,
                    tc.tile_pool(name="atp", bufs=1) as atp,
                    tc.tile_pool(name="asp", bufs=1) as asp,
                    tc.tile_pool(name="osp", bufs=2) as osp,
                ):
                    def do_av(av_pair, hp, jg, ats):
                        for u in range(2):
                            j = 2 * jg + u
                            usl = slice(u * NT, (u + 1) * NT)
                            for idx in range(2):
                                h = 2 * hp + idx
                                nc.tensor.matmul(
                                    av_pair[idx][: HD + 1, :],
                                    vS[:, j, h * 65 : h * 65 + 65],
                                    ats[idx][:, usl],
                                    start=(j == 0), stop=(j == NCH - 1),
                                    skip_group_check=True,
                                )

                    from collections import deque
                    pending = deque()  # PE work paced into the next sweeps
                    carry = [None]     # previous sweep's deferred finisher

                    def pace(k):
                        for _ in range(k):
                            if pending:
                                pending.popleft()()

                    for nt in range(N // NT):
                        nsl = slice(nt * NT, (nt + 1) * NT)
                        avc_l = []  # bf16 sbuf copies
                        rds_l = []  # f32 reciprocal rows
                        ot_l = [None, None]
                        st = {}

                        def mk_avp(hp, avc_l=avc_l, st=st):
                            def run():
                                avp = T(paux, [128, NT], "aux", bufs=2)
                                for q in range(2):
                                    nc.tensor.matmul(
                                        avp, jmat_sb[:, q], avc_l[2 * hp + q],
                                        start=(q == 0), stop=(q == 1),
                                    )
                                st[("avp", hp)] = avp
                            return run

                        def mk_bc(hp, rds_l=rds_l, ot_l=ot_l, st=st):
                            def run():
                                bcp = T(paux, [128, NT], "aux", bufs=2)
                                for q in range(2):
                                    nc.tensor.matmul(
                                        bcp,
                                        selr_sb[HD : HD + 1, q],
                                        rds_l[2 * hp + q][HD : HD + 1],
                                        start=(q == 0), stop=(q == 1),
                                        tile_position=(HD, 0),
                                    )
                                bcs = T(asp, [128, NT], "bcs", bufs=2)
                                nc.vector.tensor_copy(out=bcs, in_=bcp)
                                ot = T(asp, [128, NT], "ot", bufs=3, dt=BF16)
                                nc.vector.tensor_mul(ot, st[("avp", hp)], bcs)
                                ot_l[hp] = ot
                            return run

                        def mk_po(oc, ot_l=ot_l, st=st):
                            def run():
                                po = T(paux, [128, NT], "aux", bufs=2)
                                for p in range(2):
                                    nc.tensor.matmul(
                                        po,
                                        wproj_sb[:, p, oc * 128 : (oc + 1) * 128],
                                        ot_l[p],
                                        start=(p == 0), stop=(p == 1),
                                    )
                                st[("po", oc)] = po
                            return run

                        last_nt = nt == N // NT - 1

                        def mk_poa(oc, ot_l=ot_l, st=st):
                            # pair-0 half of the out-proj, shipped out through
                            # outT2 during the pair-1 sweep; the host adds it
                            def run():
                                poa = T(paux, [128, NT], "aux", bufs=2)
                                nc.tensor.matmul(
                                    poa,
                                    wproj_sb[:, 0, oc * 128 : (oc + 1) * 128],
                                    ot_l[0],
                                    start=True, stop=True,
                                )
                                pas = T(asp, [128, NT], "poa", bufs=4, dt=F16)
                                nc.vector.tensor_copy(out=pas, in_=poa)
                                nc.sync.dma_start(
                                    outT2.rearrange("(o p) f -> p o f", p=128)[
                                        :, oc
                                    ],
                                    pas,
                                )
                            return run

                        def mk_ob(ocp, nsl=nsl, st=st, use_act=last_nt):
                            def run():
                                ob = T(osp, [128, 2, NT], "ob", dt=F16)
                                if use_act:
                                    # tail flush: act engine is idle there
                                    nc.scalar.activation(
                                        ob[:, 0], st[("po", 2 * ocp)], AF.Copy
                                    )
                                    nc.scalar.activation(
                                        ob[:, 1], st[("po", 2 * ocp + 1)], AF.Copy
                                    )
                                else:
                                    nc.vector.tensor_copy(
                                        ob[:, 0], st[("po", 2 * ocp)]
                                    )
                                    nc.vector.tensor_copy(
                                        ob[:, 1], st[("po", 2 * ocp + 1)]
                                    )
                                nc.sync.dma_start(
                                    outT.rearrange("(o p) f -> p o f", p=128)[
                                        :, 2 * ocp : 2 * ocp + 2, nsl
                                    ],
                                    ob,
                                )
                            return run

                        def mk_finisher(av_pair, hp, prev, avc_l=avc_l,
                                        rds_l=rds_l, ot_l=ot_l, nsl=nsl,
                                        last_nt=last_nt, mk_avp=mk_avp,
                                        mk_bc=mk_bc, mk_po=mk_po, mk_ob=mk_ob,
                                        mk_poa=mk_poa):
                            # Runs after the NEXT sweep's first QK/exp so the
                            # act engine never waits on the flush at sweep
                            # boundaries.
                            def fin():
                                do_av(av_pair, hp, prev[0], prev[1])
                                tail_pair = last_nt and hp == 1
                                for idx in range(2):
                                    # reciprocal first: it gates the bc matmul
                                    rd = T(asp, [HD + 1, NT], "rds", bufs=4,
                                           dt=R32)
                                    with nc.allow_low_precision(
                                        reason="float32r output is fp32 storage"
                                    ):
                                        nc.vector.reciprocal(
                                            rd[HD : HD + 1],
                                            av_pair[idx][HD : HD + 1, :],
                                        )
                                    rds_l.append(rd)
                                    ac = T(asp, [HD + 1, NT], "avc", bufs=4,
                                           dt=BF16)
                                    if tail_pair:
                                        nc.scalar.activation(
                                            ac, av_pair[idx][: HD + 1, :],
                                            AF.Copy,
                                        )
                                    else:
                                        nc.vector.tensor_copy(
                                            out=ac,
                                            in_=av_pair[idx][: HD + 1, :],
                                        )
                                    avc_l.append(ac)
                                if hp == 0:
                                    # pair-0 normalize runs in pair-1's sweep
                                    pending.append(mk_avp(0))
                                    pending.append(mk_bc(0))
                                    if last_nt:
                                        for oc in range(8):
                                            pending.append(mk_poa(oc))
                                elif not last_nt:
                                    pending.append(mk_avp(1))
                                    pending.append(mk_bc(1))
                                    for oc in range(8):
                                        pending.append(mk_po(oc))
                                        if oc % 2 == 1:
                                            pending.append(mk_ob(oc // 2))
                                else:
                                    # final tile tail: pair-0 half of the
                                    # out-proj already left via outT2; only
                                    # the pair-1 half remains here
                                    while pending:
                                        pending.popleft()()
                                    mk_avp(1)()
                                    mk_bc(1)()
                                    for ocp in range(4):
                                        obs = T(osp, [128, 2, NT], "ob",
                                                dt=F16)
                                        for s in range(2):
                                            oc = 2 * ocp + s
                                            po = T(paux, [128, NT], "aux",
                                                   bufs=2)
                                            nc.tensor.matmul(
                                                po,
                                                wproj_sb[
                                                    :, 1,
                                                    oc * 128 : (oc + 1) * 128,
                                                ],
                                                ot_l[1],
                                                start=True, stop=True,
                                            )
                                            # alternate drain engines
                                            if s == 0:
                                                nc.scalar.activation(
                                                    obs[:, s], po, AF.Copy
                                                )
                                            else:
                                                nc.vector.tensor_copy(
                                                    out=obs[:, s], in_=po
                                                )
                                        nc.sync.dma_start(
                                            outT.rearrange(
                                                "(o p) f -> p o f", p=128
                                            )[:, 2 * ocp : 2 * ocp + 2, nsl],
                                            obs,
                                        )
                            return fin

                        for hp in range(2):
                            av_pair = [T(pav, [128, NT], "av", bufs=2)
                                       for _ in range(2)]
                            prev = None
                            for jg in range(9):  # 2-chunk batches, AV lags 1
                                pl = [T(pbig, [128, 2 * NT], "big", bufs=2)
                                      for _ in range(2)]
                                for u in range(2):
                                    j = 2 * jg + u
                                    for idx in range(2):
                                        prt = slice(64 * idx, 64 * idx + 64)
                                        nc.tensor.matmul(
                                            pl[idx][:, u * NT : (u + 1) * NT],
                                            kT[prt, hp, j * 128 : (j + 1) * 128],
                                            qT[prt, hp, nsl],
                                            start=True, stop=True,
                                            tile_position=(64 * idx, 0),
                                        )
                                ats = []
                                for idx in range(2):
                                    at = T(atp, [128, 2 * NT], "at", bufs=6,
                                           dt=BF16)
                                    nc.scalar.activation(
                                        at, pl[idx], AF.Exp,
                                        bias=zero_sb[:], scale=SCALE,
                                    )
                                    ats.append(at)
                                if jg == 0 and carry[0] is not None:
                                    carry[0]()
                                    carry[0] = None
                                pace(2 if (last_nt and hp == 1) else 1)
                                if prev is not None:
                                    do_av(av_pair, hp, prev[0], prev[1])
                                prev = (jg, ats)
                            carry[0] = mk_finisher(av_pair, hp, prev)
                    if carry[0] is not None:
                        carry[0]()
                        carry[0] = None
                    while pending:
                        pending.popleft()()
    _orig = bacc.get_activation_tables

    def _tables(arch):
        t = _orig(arch)
        return {
            name: (set() if name in ("exp_and_others", "natural_log",
                                     "exp_and_friends") else fns)
            for name, fns in t.items()
        }

    bacc.get_activation_tables = _tables
    try:
        nc.compile()
    finally:
        bacc.get_activation_tables = _orig
    return nc


_PROGRAM = None


def _get_program():
    global _PROGRAM
    if _PROGRAM is None:
        _PROGRAM = build_program()
    return _PROGRAM


def _make_in_maps(x, y, qkv_w, qkv_b, kv_w, kv_b, qn_w, kn_w, proj_w, proj_b):
    import ml_dtypes

    f = np.float32
    bf = ml_dtypes.bfloat16
    onesb = np.zeros((128, 2), f)
    onesb[0:64, 0] = 1.0
    onesb[64:128, 1] = 1.0
    w2qk = np.zeros((2, 2, 128), f)
    for wsel, w in ((0, qn_w), (1, kn_w)):
        w2qk[0, wsel, 0:64] = w
        w2qk[1, wsel, 64:128] = w
    w2qk = w2qk.reshape(2, 256)
    jmat = np.zeros((HD + 1, 2, 128), f)
    for d in range(HD):
        jmat[d, 0, d] = 1.0
        jmat[d, 1, 64 + d] = 1.0
    jmat = jmat.reshape(HD + 1, 256).astype(bf)
    selr = np.zeros((HD + 1, 2, 128), f)
    selr[HD, 0, 0:64] = 1.0
    selr[HD, 1, 64:128] = 1.0
    selr = selr.reshape(HD + 1, 256)

    def _vext(w, b):
        """[256, C] weights + [256] bias -> [260, C] / [260] with per-head
        ones column (zero w row, bias 1.0)."""
        we = np.zeros((VW, C), f)
        be = np.zeros((VW,), f)
        for h in range(HPC):
            we[65 * h : 65 * h + 64] = w[64 * h : 64 * h + 64]
            be[65 * h : 65 * h + 64] = b[64 * h : 64 * h + 64]
            be[65 * h + 64] = 1.0
        return we, be

    in_maps = []
    for core in range(8):
        b, g = divmod(core, 4)
        qs = slice(g * 256, (g + 1) * 256)
        wqkv = np.concatenate([qkv_w[qs], qkv_w[1024:2048][qs]], axis=0)
        bq = np.concatenate([qkv_b[qs], qkv_b[1024:2048][qs]])
        wkv = kv_w[qs]
        bk = kv_b[qs]
        wvx, bvxv = _vext(qkv_w[2048:3072][qs], qkv_b[2048:3072][qs])
        wvy, bvyv = _vext(kv_w[1024:2048][qs], kv_b[1024:2048][qs])
        wp = np.ascontiguousarray(proj_w[:, qs].T, f)  # [256, 1024]
        wproj2 = np.ascontiguousarray(
            wp.reshape(2, 128, C).transpose(1, 0, 2).reshape(128, 2 * C)
        ).astype(bf)
        in_maps.append(
            {
                "xT": np.ascontiguousarray(x[b].T).astype(bf),
                "yT": np.ascontiguousarray(y[b].T).astype(bf),
                "wqkvT": np.ascontiguousarray(wqkv.T).astype(bf),
                "bqkvr": bq.reshape(1, 512).astype(bf),
                "wkvT": np.ascontiguousarray(wkv.T).astype(bf),
                "bkvr": bk.reshape(1, 256).astype(bf),
                "wvxT": np.ascontiguousarray(wvx.T).astype(bf),
                "bvx": np.ascontiguousarray(bvxv.reshape(1, VW), f),
                "wvyT": np.ascontiguousarray(wvy.T).astype(bf),
                "bvy": np.ascontiguousarray(bvyv.reshape(1, VW), f),
                "ones1r": np.ones((1, 128), f),
                "wproj2": wproj2,
                "onesb": onesb,
                "w2qk": w2qk,
                "jmat": jmat,
                "selr": selr,
            }
        )
    return in_maps


def run_cores(inputs, trace=False, **kwargs):
    nc = _get_program()
    in_maps = _make_in_maps(**{k: np.asarray(v, np.float32) for k, v in inputs.items()})
    return run_bass_kernel_spmd(
        nc, in_maps, core_ids=list(range(8)), trace=trace, **kwargs
    )


def kernel(**inputs):
    proj_b = np.asarray(inputs["proj_b"], np.float32)
    res = run_cores(inputs).results
    out = np.zeros((B, N, C), np.float32)
    for core in range(8):
        b = core // 4
        out[b] += res[core]["outT"].T.astype(np.float32)
        # pair-0 half of the final query tile, shipped separately
        out[b, N - NT : N] += res[core]["outT2"].T.astype(np.float32)
    out += proj_b[None, None, :]
    return out
